# revision 16
# baseline (speedup 1.0000x reference)
"""PSROIPool Trainium2 kernel (8-core SPMD, data-parallel over ROIs/images).

Design:
  - Host: sort ROIs by batch index, cut into 8 chunks of 256. Each chunk
    touches at most 2 images (verified; fallback pads to 3 tiles with
    batch-grouped assignment). Host also precomputes, per ROI, the 0/1
    interval masks for the pooling bins (tiny, rois-derived data only).
  - Device (per core): feat tile [128=(slot,y), 245*64=(ch,x)] in SBUF.
    Stage A (PE): for each (ph, roi-tile): matmul with lhsT = combined
    (batch-onehot x mh) mask [128=(slot,y), 128=rois] contracting y,
    rhs = feat slice [(slot,y), (c,pw,x)] -> PSUM [128 rois, (c,pw,x)].
    Runs in float32r (falls back to float32 via USE_F32R=0).
    Stage B (DVE): multiply by mw mask [rois, (pw,x)], reduce over x,
    scale by 1/bin_area, store [rois, (c,ph,pw)].
  - Gather: host scatters per-core outputs back to [2048, 5, 7, 7].
"""

import os
from contextlib import ExitStack

import numpy as np

import concourse.bass as bass
import concourse.bacc as bacc
import concourse.mybir as mybir
import concourse.tile as tile
from concourse.bass_utils import run_bass_kernel_spmd

# Problem constants (hardcoded per spec).
N_IMG = 8
OD = 5          # output dim
GS = 7          # group size == pooled h/w
C = OD * GS * GS  # 245
H = W = 64
R = 2048
SS = 1.0 / 16.0
N_CORES = 8
F32 = mybir.dt.float32
F32R = mybir.dt.float32r

USE_F32R = os.environ.get("PSROI_USE_F32R", "0") == "1"

_NC_CACHE: dict = {}


def _build_nc(rt: int, reps: int = 1):
    """Build the SPMD Bass program. rt = number of 128-roi tiles per core.
    reps > 1 repeats the whole pipeline (for slope-based timing)."""
    nc = bacc.Bacc()
    cap = rt * 128
    chx = C * W  # 15680

    feat2 = nc.declare_dram_parameter("feat2", [128, chx], F32, isOutput=False)
    mh = nc.declare_dram_parameter("mh", [128, rt * GS * 128], F32, isOutput=False)
    mw = nc.declare_dram_parameter("mw", [128, rt * GS * W], F32, isOutput=False)
    outp = nc.declare_dram_parameter("out", [128, rt * C], F32, isOutput=True)

    mm_dt = F32R if USE_F32R else F32

    with tile.TileContext(nc) as tc:
        with ExitStack() as ctx:
            pool = ctx.enter_context(tc.tile_pool(name="sb", bufs=1 if reps == 1 else 2))
            stp = ctx.enter_context(tc.tile_pool(name="stg", bufs=3))
            psp = ctx.enter_context(
                tc.tile_pool(name="ps", bufs=8, space=bass.MemorySpace.PSUM)
            )

            for _rep in range(reps):
                featT = pool.tile([128, chx], F32, tag="feat")
                nc.sync.dma_start(featT[:], feat2[:])
                mhT = pool.tile([128, rt * GS * 128], F32, tag="mh")
                nc.sync.dma_start(mhT[:], mh[:])
                mwT = pool.tile([128, rt * GS * W], F32, tag="mw")
                nc.sync.dma_start(mwT[:], mw[:])
                outT = pool.tile([128, rt * C], F32, tag="out")

                # feat free-dim layout: (c, g=(ph,pw), x)
                featv = featT[:].rearrange("p (c g x) -> p c g x", c=OD, g=GS * GS, x=W)
                outv = outT[:].rearrange("p (t c h w) -> p t c h w", t=rt, c=OD, h=GS)


                for t in range(rt):
                    for ph in range(GS):
                        lhs = mhT[:, (t * GS + ph) * 128 : (t * GS + ph + 1) * 128]
                        stg = stp.tile([128, OD * GS * W], F32, tag="stg")  # [128, 2240]
                        for c in range(OD):
                            ps = psp.tile([128, GS * W], F32, tag="ps")  # [128, 448]
                            rhs = featv[:, c, ph * GS : (ph + 1) * GS, :]
                            nc.tensor.matmul(
                                ps[:],
                                lhs.bitcast(mm_dt),
                                rhs.bitcast(mm_dt),
                                start=True,
                                stop=True,
                            )
                            nc.vector.tensor_mul(
                                stg[:, c * (GS * W) : (c + 1) * (GS * W)],
                                ps[:],
                                mwT[:, t * (GS * W) : (t + 1) * (GS * W)],
                            )
                        stgv = stg[:].rearrange("p (cq x) -> p cq x", x=W)
                        # masks carry 1/count; reduce writes the final values
                        nc.vector.reduce_sum(
                            outv[:, t, :, ph, :], stgv, axis=mybir.AxisListType.X
                        )

                nc.sync.dma_start(outp[:], outT[:])

    nc.finalize()
    return nc


def _get_nc(rt: int, reps: int = 1):
    key = (rt, reps)
    if key not in _NC_CACHE:
        _NC_CACHE[key] = _build_nc(rt, reps)
    return _NC_CACHE[key]


def _bin_bounds(rois: np.ndarray):
    """Replicates the reference's fp32 bin-boundary math exactly (numpy)."""
    f = np.float32
    rois = rois.astype(f)
    xs = np.round(rois[:, 1]) * f(SS)
    ys = np.round(rois[:, 2]) * f(SS)
    xe = np.round(rois[:, 3] + f(1.0)) * f(SS)
    ye = np.round(rois[:, 4] + f(1.0)) * f(SS)
    roi_w = np.maximum(xe - xs, f(0.1))
    roi_h = np.maximum(ye - ys, f(0.1))
    # This platform's jax lowers x/7 to x * round32(1/7); replicate exactly.
    inv_gs = f(1.0) / f(GS)
    bin_w = (roi_w * inv_gs).astype(f)
    bin_h = (roi_h * inv_gs).astype(f)
    pidx = np.arange(GS, dtype=f)
    hstart = np.clip(np.floor(pidx[None, :] * bin_h[:, None] + ys[:, None]), 0, H)
    hend = np.clip(np.ceil((pidx[None, :] + f(1.0)) * bin_h[:, None] + ys[:, None]), 0, H)
    wstart = np.clip(np.floor(pidx[None, :] * bin_w[:, None] + xs[:, None]), 0, W)
    wend = np.clip(np.ceil((pidx[None, :] + f(1.0)) * bin_w[:, None] + xs[:, None]), 0, W)
    return hstart, hend, wstart, wend


def _shard(rois: np.ndarray):
    """Assign ROIs to cores. Returns (chunks[core] -> roi idx array, rt,
    images[core] -> (iA, iB))."""
    batch = rois[:, 0].astype(np.int32)
    order = np.argsort(batch, kind="stable")
    if R % N_CORES == 0:
        chunks = [order[i * (R // N_CORES) : (i + 1) * (R // N_CORES)] for i in range(N_CORES)]
        if all(len(np.unique(batch[c])) <= 2 for c in chunks):
            return chunks, (R // N_CORES + 127) // 128, batch, chunks
    # Fallback: group by batch (one image per core), pad capacity.
    chunks = [np.nonzero(batch == i)[0] for i in range(N_CORES)]
    maxc = max(len(c) for c in chunks)
    rt = (maxc + 127) // 128
    return chunks, rt, batch, chunks


def _run_cores(feat: np.ndarray, rois: np.ndarray, trace: bool = False, reps: int = 1):
    feat = np.ascontiguousarray(np.asarray(feat, dtype=np.float32))
    rois = np.asarray(rois, dtype=np.float32)
    assert feat.shape == (N_IMG, C, H, W), feat.shape
    assert rois.shape == (R, 5), rois.shape

    chunks, rt, batch, _ = _shard(rois)
    cap = rt * 128
    nc = _get_nc(rt, reps)

    hs, he, ws, we = _bin_bounds(rois)
    cnt_h = (he - hs).astype(np.float32)
    cnt_w = (we - ws).astype(np.float32)
    inv_h = np.where(cnt_h > 0, np.float32(1.0) / np.maximum(cnt_h, 1), 0).astype(np.float32)
    inv_w = np.where(cnt_w > 0, np.float32(1.0) / np.maximum(cnt_w, 1), 0).astype(np.float32)

    yi = np.arange(H, dtype=np.float32)
    xi = np.arange(W, dtype=np.float32)
    # [R, GS, H/W] interval masks with 1/count folded in
    mask_h = ((yi[None, None, :] >= hs[:, :, None]) & (yi[None, None, :] < he[:, :, None])).astype(np.float32)
    mask_h *= inv_h[:, :, None]
    mask_w = ((xi[None, None, :] >= ws[:, :, None]) & (xi[None, None, :] < we[:, :, None])).astype(np.float32)
    mask_w *= inv_w[:, :, None]

    in_maps = []
    for core in range(N_CORES):
        idx = chunks[core]
        n_r = len(idx)
        imgs = np.unique(batch[idx])
        assert len(imgs) <= 2, f"core {core} spans {len(imgs)} images"
        iA = int(imgs[0])
        iB = int(imgs[1]) if len(imgs) > 1 else iA
        slot = (batch[idx] == iB).astype(np.int64) if iB != iA else np.zeros(n_r, np.int64)

        fpair = feat[[iA, iB]]  # [2, C, H, W]
        feat2 = np.ascontiguousarray(
            fpair.transpose(0, 2, 1, 3).reshape(128, C * W)
        )

        rr = np.arange(n_r)
        rt_idx = rr // 128
        rp_idx = rr % 128

        # mh: [(slot,y) part, (t, ph, rp)]
        mh_t = np.zeros((rt, 128, 2, GS, H), np.float32)  # [t, rp, slot, ph, y]
        mh_t[rt_idx, rp_idx, slot] = mask_h[idx]
        mh_host = np.ascontiguousarray(
            mh_t.transpose(2, 4, 0, 3, 1).reshape(128, rt * GS * 128)
        )

        # mw: [rp part, (t, pw, x)]
        mw_t = np.zeros((rt, 128, GS, W), np.float32)
        mw_t[rt_idx, rp_idx] = mask_w[idx]
        mw_host = np.ascontiguousarray(
            mw_t.transpose(1, 0, 2, 3).reshape(128, rt * GS * W)
        )

        in_maps.append({"feat2": feat2, "mh": mh_host, "mw": mw_host})

    res = run_bass_kernel_spmd(nc, in_maps, list(range(N_CORES)), trace=trace)

    out_full = np.zeros((R, OD, GS, GS), np.float32)
    for core in range(N_CORES):
        idx = chunks[core]
        o = np.asarray(res.results[core]["out"])  # [128, rt*C]
        o = o.reshape(128, rt, OD, GS, GS).transpose(1, 0, 2, 3, 4).reshape(cap, OD, GS, GS)
        out_full[idx] = o[: len(idx)]
    return out_full, res


def kernel(feat: np.ndarray, rois: np.ndarray) -> np.ndarray:
    out, _ = _run_cores(feat, rois, trace=False)
    return out


# revision 23
# speedup vs baseline: 348.4593x; 348.4593x over previous
"""PSROIPool Trainium2 kernel (8-core SPMD, data-parallel over ROIs/images).

Design:
  - Host: sort ROIs by batch index, cut into 8 chunks of 256. Each chunk
    touches at most 2 images (verified; fallback pads to 3 tiles with
    batch-grouped assignment). Host also precomputes, per ROI, the 0/1
    interval masks for the pooling bins (tiny, rois-derived data only).
  - Device (per core): feat tile [128=(slot,y), 245*64=(ch,x)] in SBUF.
    Stage A (PE, float32r at 1 cyc/row): for each (ph, roi-tile, c-pair):
    matmul with lhsT = combined (batch-onehot x mh/count_h) mask
    [128=(slot,y), 128=rois] contracting y, rhs = feat slice
    [(slot,y), (pw,x)] -> PSUM [128 rois, (pw,x)] (bank-aligned pairs).
    Stage B: DVE multiplies by the mw/count_w mask (1/bin_area is folded
    into the two masks); the 35-segment x-reduction is split between DVE
    (reduce_sum) and the otherwise-idle ScalarE (activation accum_out),
    writing final [rois, (c,ph,pw)] directly.
  - Gather: host scatters per-core outputs back to [2048, 5, 7, 7].
  Measured on HW: ~51 us/core kernel body, rel err ~1.4e-4 (fp32r
  rounding; PSROI_USE_F32R=0 gives exact fp32 at ~75 us).
"""

import os
from contextlib import ExitStack

import numpy as np

import concourse.bass as bass
import concourse.bacc as bacc
import concourse.mybir as mybir
import concourse.tile as tile
from concourse.bass_utils import run_bass_kernel_spmd

# Problem constants (hardcoded per spec).
N_IMG = 8
OD = 5          # output dim
GS = 7          # group size == pooled h/w
C = OD * GS * GS  # 245
H = W = 64
R = 2048
SS = 1.0 / 16.0
N_CORES = 8
F32 = mybir.dt.float32
F32R = mybir.dt.float32r

USE_F32R = os.environ.get("PSROI_USE_F32R", "1") == "1"
ACT_COMBOS = int(os.environ.get("PSROI_ACT_COMBOS", "7"))

_NC_CACHE: dict = {}


def _build_nc(rt: int, reps: int = 1):
    """Build the SPMD Bass program. rt = number of 128-roi tiles per core.
    reps > 1 repeats the whole pipeline (for slope-based timing)."""
    nc = bacc.Bacc()
    cap = rt * 128
    chx = C * W  # 15680

    mm_dt = F32R if USE_F32R else F32
    feat2 = nc.declare_dram_parameter("feat2", [128, chx], mm_dt, isOutput=False)
    mh = nc.declare_dram_parameter("mh", [128, rt * GS * 128], mm_dt, isOutput=False)
    mw = nc.declare_dram_parameter("mw", [128, rt * GS * W], F32, isOutput=False)
    outp = nc.declare_dram_parameter("out", [128, rt * C], F32, isOutput=True)

    with tile.TileContext(nc) as tc:
        with ExitStack() as ctx:
            pool = ctx.enter_context(tc.tile_pool(name="sb", bufs=1 if reps == 1 else 2))
            stp = ctx.enter_context(tc.tile_pool(name="stg", bufs=3))
            psp = ctx.enter_context(
                tc.tile_pool(name="ps", bufs=4, space=bass.MemorySpace.PSUM)
            )

            for _rep in range(reps):
                featT = pool.tile([128, chx], mm_dt, tag="feat")
                nc.sync.dma_start(featT[:], feat2[:])
                mhT = pool.tile([128, rt * GS * 128], mm_dt, tag="mh")
                nc.sync.dma_start(mhT[:], mh[:])
                mwT = pool.tile([128, rt * GS * W], F32, tag="mw")
                nc.sync.dma_start(mwT[:], mw[:])
                outT = pool.tile([128, rt * C], F32, tag="out")

                # feat free-dim layout: (c, g=(ph,pw), x)
                featv = featT[:].rearrange("p (c g x) -> p c g x", c=OD, g=GS * GS, x=W)
                outv = outT[:].rearrange("p (t c h w) -> p t c h w", t=rt, c=OD, h=GS)

                scr = pool.tile([128, W], F32, tag="scr")  # ACT scratch

                n_combo = rt * GS
                for t in range(rt):
                    for ph in range(GS):
                        k = t * GS + ph
                        lhs = mhT[:, k * 128 : (k + 1) * 128]
                        stg = stp.tile([128, OD * GS * W], F32, tag="stg")  # [128, 2240]
                        # c-chunks of 2 (2-bank psum tiles) to amortize the
                        # PSUM read bubble on the DVE multiply.
                        for c0 in range(0, OD, 2):
                            ncc = min(2, OD - c0)
                            # bank-aligned per-c regions (512 fp32 = 1 bank)
                            ps = psp.tile([128, ncc * 512], F32, tag="ps")
                            psv = ps[:].rearrange("p (i b) -> p i b", b=512)
                            for i in range(ncc):
                                c = c0 + i
                                rhs = featv[:, c, ph * GS : (ph + 1) * GS, :]
                                nc.tensor.matmul(
                                    psv[:, i, 0 : GS * W],
                                    lhs,
                                    rhs,
                                    start=True,
                                    stop=True,
                                )
                            mwv = mwT[:, t * (GS * W) : (t + 1) * (GS * W)]
                            stgw = stg[:, c0 * (GS * W) : (c0 + ncc) * (GS * W)]
                            if ncc > 1:
                                nc.vector.tensor_mul(
                                    stgw.rearrange("p (i q) -> p i q", i=ncc),
                                    psv[:, :, 0 : GS * W],
                                    mwv.unsqueeze(1).broadcast_to(
                                        [128, ncc, GS * W]
                                    ),
                                )
                            else:
                                nc.vector.tensor_mul(
                                    stgw, psv[:, 0, 0 : GS * W], mwv
                                )
                        stgv = stg[:].rearrange("p (cq x) -> p cq x", x=W)
                        # Split the 35-segment reduction: DVE handles some
                        # combos, ScalarE (otherwise idle) the rest.
                        if k % 2 == 1 and (k // 2) < ACT_COMBOS:
                            stgq = stg[:].rearrange(
                                "p (c q x) -> p c q x", c=OD, q=GS
                            )
                            for c in range(OD):
                                for q in range(GS):
                                    nc.scalar.activation(
                                        scr[:],
                                        stgq[:, c, q, :],
                                        mybir.ActivationFunctionType.Copy,
                                        accum_out=outv[:, t, c, ph, q : q + 1],
                                    )
                        else:
                            nc.vector.reduce_sum(
                                outv[:, t, :, ph, :], stgv, axis=mybir.AxisListType.X
                            )

                nc.sync.dma_start(outp[:], outT[:])

    nc.finalize()
    return nc


def _get_nc(rt: int, reps: int = 1):
    key = (rt, reps)
    if key not in _NC_CACHE:
        _NC_CACHE[key] = _build_nc(rt, reps)
    return _NC_CACHE[key]


def _bin_bounds(rois: np.ndarray):
    """Replicates the reference's fp32 bin-boundary math exactly (numpy)."""
    f = np.float32
    rois = rois.astype(f)
    xs = np.round(rois[:, 1]) * f(SS)
    ys = np.round(rois[:, 2]) * f(SS)
    xe = np.round(rois[:, 3] + f(1.0)) * f(SS)
    ye = np.round(rois[:, 4] + f(1.0)) * f(SS)
    roi_w = np.maximum(xe - xs, f(0.1))
    roi_h = np.maximum(ye - ys, f(0.1))
    # This platform's jax lowers x/7 to x * round32(1/7); replicate exactly.
    inv_gs = f(1.0) / f(GS)
    bin_w = (roi_w * inv_gs).astype(f)
    bin_h = (roi_h * inv_gs).astype(f)
    pidx = np.arange(GS, dtype=f)
    hstart = np.clip(np.floor(pidx[None, :] * bin_h[:, None] + ys[:, None]), 0, H)
    hend = np.clip(np.ceil((pidx[None, :] + f(1.0)) * bin_h[:, None] + ys[:, None]), 0, H)
    wstart = np.clip(np.floor(pidx[None, :] * bin_w[:, None] + xs[:, None]), 0, W)
    wend = np.clip(np.ceil((pidx[None, :] + f(1.0)) * bin_w[:, None] + xs[:, None]), 0, W)
    return hstart, hend, wstart, wend


def _shard(rois: np.ndarray):
    """Assign ROIs to cores. Returns (chunks[core] -> roi idx array, rt,
    images[core] -> (iA, iB))."""
    batch = rois[:, 0].astype(np.int32)
    order = np.argsort(batch, kind="stable")
    if R % N_CORES == 0:
        chunks = [order[i * (R // N_CORES) : (i + 1) * (R // N_CORES)] for i in range(N_CORES)]
        if all(len(np.unique(batch[c])) <= 2 for c in chunks):
            return chunks, (R // N_CORES + 127) // 128, batch, chunks
    # Fallback: group by batch (one image per core), pad capacity.
    chunks = [np.nonzero(batch == i)[0] for i in range(N_CORES)]
    maxc = max(len(c) for c in chunks)
    rt = (maxc + 127) // 128
    return chunks, rt, batch, chunks


def _run_cores(feat: np.ndarray, rois: np.ndarray, trace: bool = False, reps: int = 1):
    feat = np.ascontiguousarray(np.asarray(feat, dtype=np.float32))
    rois = np.asarray(rois, dtype=np.float32)
    assert feat.shape == (N_IMG, C, H, W), feat.shape
    assert rois.shape == (R, 5), rois.shape

    chunks, rt, batch, _ = _shard(rois)
    cap = rt * 128
    nc = _get_nc(rt, reps)

    hs, he, ws, we = _bin_bounds(rois)
    cnt_h = (he - hs).astype(np.float32)
    cnt_w = (we - ws).astype(np.float32)
    inv_h = np.where(cnt_h > 0, np.float32(1.0) / np.maximum(cnt_h, 1), 0).astype(np.float32)
    inv_w = np.where(cnt_w > 0, np.float32(1.0) / np.maximum(cnt_w, 1), 0).astype(np.float32)

    yi = np.arange(H, dtype=np.float32)
    xi = np.arange(W, dtype=np.float32)
    # [R, GS, H/W] interval masks with 1/count folded in
    mask_h = ((yi[None, None, :] >= hs[:, :, None]) & (yi[None, None, :] < he[:, :, None])).astype(np.float32)
    mask_h *= inv_h[:, :, None]
    mask_w = ((xi[None, None, :] >= ws[:, :, None]) & (xi[None, None, :] < we[:, :, None])).astype(np.float32)
    mask_w *= inv_w[:, :, None]

    in_maps = []
    for core in range(N_CORES):
        idx = chunks[core]
        n_r = len(idx)
        imgs = np.unique(batch[idx])
        assert len(imgs) <= 2, f"core {core} spans {len(imgs)} images"
        iA = int(imgs[0])
        iB = int(imgs[1]) if len(imgs) > 1 else iA
        slot = (batch[idx] == iB).astype(np.int64) if iB != iA else np.zeros(n_r, np.int64)

        fpair = feat[[iA, iB]]  # [2, C, H, W]
        feat2 = np.ascontiguousarray(
            fpair.transpose(0, 2, 1, 3).reshape(128, C * W)
        )

        rr = np.arange(n_r)
        rt_idx = rr // 128
        rp_idx = rr % 128

        # mh: [(slot,y) part, (t, ph, rp)]
        mh_t = np.zeros((rt, 128, 2, GS, H), np.float32)  # [t, rp, slot, ph, y]
        mh_t[rt_idx, rp_idx, slot] = mask_h[idx]
        mh_host = np.ascontiguousarray(
            mh_t.transpose(2, 4, 0, 3, 1).reshape(128, rt * GS * 128)
        )

        # mw: [rp part, (t, pw, x)]
        mw_t = np.zeros((rt, 128, GS, W), np.float32)
        mw_t[rt_idx, rp_idx] = mask_w[idx]
        mw_host = np.ascontiguousarray(
            mw_t.transpose(1, 0, 2, 3).reshape(128, rt * GS * W)
        )

        in_maps.append({"feat2": feat2, "mh": mh_host, "mw": mw_host})

    res = run_bass_kernel_spmd(nc, in_maps, list(range(N_CORES)), trace=trace)

    out_full = np.zeros((R, OD, GS, GS), np.float32)
    for core in range(N_CORES):
        idx = chunks[core]
        o = np.asarray(res.results[core]["out"])  # [128, rt*C]
        o = o.reshape(128, rt, OD, GS, GS).transpose(1, 0, 2, 3, 4).reshape(cap, OD, GS, GS)
        out_full[idx] = o[: len(idx)]
    return out_full, res


def kernel(feat: np.ndarray, rois: np.ndarray) -> np.ndarray:
    out, _ = _run_cores(feat, rois, trace=False)
    return out
